# revision 26
# baseline (speedup 1.0000x reference)
"""Trainium2 Bass kernel for nn_NegativeHardestContrastiveLoss.

Math: d[m, n] = ||a_m||^2 + ||b_n||^2 - 2 a_m.b_n  over m in [64] anchors,
n in [262144] pixels; loss = mean over m of mean of 5 smallest relu(d[m, :]).

Per-row ordering of d is independent of ||a_m||^2, so each core computes
neg_e = 2 a.b - ||b_n||^2 for its 1/8 slice of pixels in PSUM via 3
accumulating matmul passes (two bf16 K=128 passes with stationary 2*a, one
fp32 K=1 pass streaming the host-precomputed pixel norms against a -1
stationary), then per 2048-column PSUM group takes the top-8 largest neg_e
per row with the DVE max instruction. The host gathers the 64x128
candidates per core and reduces to the exact global top-5 in numpy.

Static DMA descriptors only have one sync-wait slot, so the dataflow keeps
every DMA at <=1 dependency: b tiles are read by PE only.
"""

import sys

for _p in ("/opt/trn_rl_repo",):
    if _p not in sys.path:
        sys.path.insert(0, _p)

import numpy as np
import ml_dtypes

C = 256
HW = 512 * 512
P = 64  # anchors
N_CORES = 8
NPC = HW // N_CORES  # pixels per core: 32768
CHUNK = 4096  # data columns per PSUM group (4 banks, 2 col-halves)
N_GROUPS = NPC // CHUNK  # 8
MM_FREE = 512
NCAND = 8 * N_GROUPS  # 64 candidates per PSUM row per core

_compiled = {}


def _build_bass(split=True):
    import concourse.bass as bass
    import concourse.mybir as mybir
    import concourse.tile as tile

    nc = bass.Bass("TRN2", debug=False)
    bf16 = mybir.dt.bfloat16
    f32 = mybir.dt.float32

    f2s = nc.dram_tensor("f2s", [C, NPC], bf16, kind="ExternalInput")
    wmat = nc.dram_tensor("wmat", [2, 128, P], bf16, kind="ExternalInput")
    # row-tiled norm layout: DRAM row 2r+k lands on partition 32r+k and
    # holds the hi/lo (k) norm rows for the chunks served by PE row-group r
    nrm = nc.dram_tensor("nrm", [8, NPC // 4], bf16, kind="ExternalInput")
    cand = nc.dram_tensor("cand", [128, NCAND], f32, kind="ExternalOutput")

    f2v = f2s.ap().rearrange("(ko pi) n -> pi ko n", pi=128)
    wv = wmat.ap().rearrange("k p m -> p k m")

    with tile.TileContext(nc) as tc:
        with (
            tc.tile_pool(name="wpool", bufs=1) as wpool,
            tc.tile_pool(name="bpool", bufs=4) as bpool,
            tc.tile_pool(name="cpool", bufs=1) as cpool,
            tc.tile_pool(name="psum", bufs=2, space="PSUM") as psum_pool,
        ):
            wt = wpool.tile([128, 2, P], bf16)
            nc.gpsimd.dma_start(wt[:], wv)
            # norm pass stationaries: -1s on every row group
            wneg = wpool.tile([128, P], bf16)
            nc.vector.memset(wneg[:], -1.0)
            nrm_sb = wpool.tile([128, NPC // 4], bf16)
            # only partitions 32r..32r+1 are ever read; load the packed DRAM
            # rows there, first two groups first (subtile deps)
            NPG = CHUNK // 4  # norm columns per group
            head = 2 * NPG
            for r in range(4):
                nc.gpsimd.dma_start(
                    nrm_sb[32 * r : 32 * r + 2, :head],
                    nrm.ap()[2 * r : 2 * r + 2, :head],
                )
            for r in range(4):
                nc.gpsimd.dma_start(
                    nrm_sb[32 * r : 32 * r + 2, head:],
                    nrm.ap()[2 * r : 2 * r + 2, head:],
                )

            cand_sb = cpool.tile([128, NCAND], f32)

            for g in range(N_GROUPS):
                bt = bpool.tile([128, 2, CHUNK], bf16)
                # split loads per K-half (subtile deps); the first group
                # loads in 1024-col pieces so the PE starts sooner
                npc = 4 if g == 0 else 1
                for ko in range(2):
                    for pc in range(npc):
                        w = CHUNK // npc
                        psl = slice(pc * w, (pc + 1) * w)
                        dsl = slice(g * CHUNK + pc * w, g * CHUNK + (pc + 1) * w)
                        nc.scalar.dma_start(bt[:, ko, psl], f2v[:, ko, dsl])

                # Each 512-wide PSUM slice holds TWO 512-col data chunks,
                # streamed concurrently through the two column halves of the
                # PE array. HW has_written clearing is per-partition (probed),
                # so the hi half piggybacks in the same bank with the
                # partition-blind sim check skipped. Stationary-major bursts
                # amortize LDWEIGHTS; the norm pass is one 8-way concurrent
                # burst of 32x64 tiles (K=2 hi/lo rows per row-group).
                ps = psum_pool.tile([128, CHUNK // 2], f32)
                NB = CHUNK // (2 * MM_FREE)  # banks per group
                for w in range(2):  # stationary w0, then w1
                    for c in range(NB):
                        pa = ps[0:P, c * MM_FREE : (c + 1) * MM_FREE]
                        pb = ps[P:128, c * MM_FREE : (c + 1) * MM_FREE]
                        lo = slice(c * 2 * MM_FREE, c * 2 * MM_FREE + MM_FREE)
                        hi = slice(c * 2 * MM_FREE + MM_FREE, (c + 1) * 2 * MM_FREE)
                        nc.tensor.matmul(
                            pa, wt[:, w, :], bt[:, w, lo],
                            start=(w == 0), stop=False,
                        )
                        nc.tensor.matmul(
                            pb, wt[:, w, :], bt[:, w, hi],
                            start=(w == 0), stop=False, skip_group_check=True,
                        )
                for c in range(NB):
                    pa = ps[0:P, c * MM_FREE : (c + 1) * MM_FREE]
                    pb = ps[P:128, c * MM_FREE : (c + 1) * MM_FREE]
                    nlo = slice(g * NPG, g * NPG + MM_FREE)
                    nhi = slice(g * NPG + MM_FREE, g * NPG + 2 * MM_FREE)
                    nc.tensor.matmul(
                        pa, wneg[32 * c : 32 * c + 2, :],
                        nrm_sb[32 * c : 32 * c + 2, nlo],
                        start=False, stop=True, tile_position=(32 * c, 0),
                    )
                    nc.tensor.matmul(
                        pb, wneg[32 * c : 32 * c + 2, :],
                        nrm_sb[32 * c : 32 * c + 2, nhi],
                        start=False, stop=True, tile_position=(32 * c, 64),
                        skip_group_check=True,
                    )

                nc.vector.max(out=cand_sb[:, g * 8 : (g + 1) * 8], in_=ps[:])

            nc.gpsimd.dma_start(cand.ap(), cand_sb[:])

    if split:
        _split_waits(nc, mybir)
    return nc


def _split_waits(nc, mybir):
    """walrus (neuronxcc) accepts at most ONE sync wait per instruction;
    hoist extra waits onto same-engine NoOps inserted just before. HWDGE
    and SWDGE waits both execute at the issuing engine's sequencer, so
    this preserves ordering."""
    k = 0
    for f in nc.m.functions:
        for bb in f.blocks:
            insts = list(bb.instructions)
            new = []
            changed = False
            for ins in insts:
                si = getattr(ins, "sync_info", None)
                if si is not None and si.on_wait and len(si.on_wait) > 1:
                    changed = True
                    waits = list(si.on_wait)
                    for w in waits[:-1]:
                        k += 1
                        nop = mybir.InstNoOp(
                            name=f"WSPLIT-{k}", engine=ins.engine,
                            ins=[], outs=[], bass_nofuse=True,
                        )
                        nop.sync_info = mybir.SyncInfo(on_wait=[w], on_update=[])
                        new.append(nop)
                    ins.sync_info = mybir.SyncInfo(
                        on_wait=[waits[-1]], on_update=list(si.on_update)
                    )
                new.append(ins)
            if changed:
                try:
                    bb.instructions = new
                except Exception:
                    bb.instructions[:] = new


def get_nc():
    if "nc" not in _compiled:
        _compiled["nc"] = _build_bass()
    return _compiled["nc"]


def _prep_inputs(feats1, feats2, positive_pairs):
    f1 = np.asarray(feats1, dtype=np.float32).reshape(C, HW)
    f2 = np.asarray(feats2, dtype=np.float32).reshape(C, HW)
    idx = np.asarray(positive_pairs)[0, :, 0].astype(np.int64)
    sel = f1[:, idx]  # [C, P] fp32 anchors
    a_norms = np.sum(sel.astype(np.float64) ** 2, axis=0)  # [P]

    w = np.empty((2, 128, P), dtype=np.float32)
    w[0] = 2.0 * sel[:128]
    w[1] = 2.0 * sel[128:]
    wmat = w.astype(ml_dtypes.bfloat16)

    b_norms = np.einsum("ij,ij->j", f2, f2).astype(np.float32)  # [HW]
    # hi/lo bf16 split so the K=2 norm pass carries ~fp32 precision
    n_hi = b_norms.astype(ml_dtypes.bfloat16)
    n_lo = (b_norms - n_hi.astype(np.float32)).astype(ml_dtypes.bfloat16)
    nrm2 = np.stack([n_hi, n_lo])  # [2, HW] bf16
    f2b = f2.astype(ml_dtypes.bfloat16)

    def nrm_rt(core):
        # row-tiled layout: partition 32r+k, pos g*1024 + cg*512 + t holds
        # nrm2[k, core_slice + g*4096 + (2r+cg)*512 + t]
        a = nrm2[:, core * NPC : (core + 1) * NPC]
        v = a.reshape(2, N_GROUPS, 4, 2, 512)  # k g r cg t
        out = np.zeros((8, NPC // 4), dtype=ml_dtypes.bfloat16)
        for r in range(4):
            for k in range(2):
                out[2 * r + k] = v[k, :, r, :, :].reshape(-1)
        return out

    in_maps = [
        {
            "f2s": np.ascontiguousarray(f2b[:, c * NPC : (c + 1) * NPC]),
            "wmat": wmat,
            "nrm": nrm_rt(c),
        }
        for c in range(N_CORES)
    ]
    return in_maps, a_norms


def _reduce_host(cands, a_norms):
    # cands: [128, N_CORES * NCAND] fp32 of neg_e = 2ab - ||b||^2 candidates;
    # anchor m's candidates live in PSUM rows m and m+64 (column-half tiling)
    merged = np.concatenate([cands[:P], cands[P:]], axis=1)  # [P, 2*...]
    d = a_norms[:, None] - merged.astype(np.float64)  # squared distances
    top5 = np.partition(d, 4, axis=1)[:, :5]
    top5 = np.maximum(top5, 0.0)
    return np.float32(np.mean(np.mean(top5, axis=1)))


def kernel(feats1, feats2, positive_pairs, **run_kwargs):
    from concourse.bass_utils import run_bass_kernel_spmd

    in_maps, a_norms = _prep_inputs(feats1, feats2, positive_pairs)
    nc = get_nc()
    res = run_bass_kernel_spmd(nc, in_maps, list(range(N_CORES)), **run_kwargs)
    cands = np.concatenate([r["cand"] for r in res.results], axis=1)
    out = _reduce_host(cands, a_norms)
    if run_kwargs:
        return out, res
    return out


# revision 32
# speedup vs baseline: 1.4310x; 1.4310x over previous
"""Trainium2 Bass kernel for nn_NegativeHardestContrastiveLoss.

Math: d[m, n] = ||a_m||^2 + ||b_n||^2 - 2 a_m.b_n  over m in [64] anchors,
n in [262144] pixels; loss = mean over m of mean of 5 smallest relu(d[m, :]).

Per-row ordering of d is independent of ||a_m||^2, so each core computes
neg_e = 2 a.b - ||b_n||^2 for its 1/8 slice of pixels in PSUM via 3
accumulating matmul passes (two bf16 K=128 passes with stationary 2*a, one
fp32 K=1 pass streaming the host-precomputed pixel norms against a -1
stationary), then per 2048-column PSUM group takes the top-8 largest neg_e
per row with the DVE max instruction. The host gathers the 64x128
candidates per core and reduces to the exact global top-5 in numpy.

Static DMA descriptors only have one sync-wait slot, so the dataflow keeps
every DMA at <=1 dependency: b tiles are read by PE only.
"""

import sys

for _p in ("/opt/trn_rl_repo",):
    if _p not in sys.path:
        sys.path.insert(0, _p)

import numpy as np
import ml_dtypes

C = 256
HW = 512 * 512
P = 64  # anchors
N_CORES = 8
NPC = HW // N_CORES  # pixels per core: 32768
CHUNK = 4096  # data columns per PSUM group (4 banks, 2 col-halves)
N_GROUPS = NPC // CHUNK  # 8
MM_FREE = 512
NCAND = 8 * N_GROUPS  # 64 candidates per PSUM row per core
FP8 = True  # stream b / anchors as fp8e4m3 (halves HBM traffic)

_compiled = {}


def _build_bass(split=True):
    import concourse.bass as bass
    import concourse.mybir as mybir
    import concourse.tile as tile

    nc = bass.Bass("TRN2", debug=False)
    bf16 = mybir.dt.bfloat16
    f32 = mybir.dt.float32
    bdt = mybir.dt.float8e4 if FP8 else bf16

    f2s = nc.dram_tensor("f2s", [C, NPC], bdt, kind="ExternalInput")
    wmat = nc.dram_tensor("wmat", [2, 128, P], bdt, kind="ExternalInput")
    # row-tiled norm layout: DRAM row 2r+k lands on partition 32r+k and
    # holds the hi/lo (k) norm rows for the chunks served by PE row-group r
    nrm = nc.dram_tensor("nrm", [8, NPC // 4], bf16, kind="ExternalInput")
    cand = nc.dram_tensor("cand", [128, NCAND], f32, kind="ExternalOutput")

    f2v = f2s.ap().rearrange("(ko pi) n -> pi ko n", pi=128)
    wv = wmat.ap().rearrange("k p m -> p k m")

    with tile.TileContext(nc) as tc:
        with (
            tc.tile_pool(name="wpool", bufs=1) as wpool,
            tc.tile_pool(name="bpool", bufs=4) as bpool,
            tc.tile_pool(name="cpool", bufs=1) as cpool,
            tc.tile_pool(name="psum", bufs=2, space="PSUM") as psum_pool,
        ):
            wt = wpool.tile([128, 2, P], bdt)
            nc.gpsimd.dma_start(wt[:], wv)
            # norm pass stationaries: -1s on every row group
            wneg = wpool.tile([128, P], bf16)
            nc.vector.memset(wneg[:], -1.0)
            nrm_sb = wpool.tile([128, NPC // 4], bf16)
            # only partitions 32r..32r+1 are ever read; load the packed DRAM
            # rows there, first two groups first (subtile deps)
            NPG = CHUNK // 4  # norm columns per group
            head = 2 * NPG
            for r in range(4):
                nc.gpsimd.dma_start(
                    nrm_sb[32 * r : 32 * r + 2, :head],
                    nrm.ap()[2 * r : 2 * r + 2, :head],
                )
            for r in range(4):
                nc.gpsimd.dma_start(
                    nrm_sb[32 * r : 32 * r + 2, head:],
                    nrm.ap()[2 * r : 2 * r + 2, head:],
                )

            cand_sb = cpool.tile([128, NCAND], f32)

            for g in range(N_GROUPS):
                bt = bpool.tile([128, 2, CHUNK], bdt)
                # split loads per K-half (subtile deps); the first group
                # loads in 1024-col pieces so the PE starts sooner
                npc = 4 if g == 0 else 1
                for ko in range(2):
                    for pc in range(npc):
                        w = CHUNK // npc
                        psl = slice(pc * w, (pc + 1) * w)
                        dsl = slice(g * CHUNK + pc * w, g * CHUNK + (pc + 1) * w)
                        nc.scalar.dma_start(bt[:, ko, psl], f2v[:, ko, dsl])

                # Each 512-wide PSUM slice holds TWO 512-col data chunks,
                # streamed concurrently through the two column halves of the
                # PE array. HW has_written clearing is per-partition (probed),
                # so the hi half piggybacks in the same bank with the
                # partition-blind sim check skipped. Stationary-major bursts
                # amortize LDWEIGHTS; the norm pass is one 8-way concurrent
                # burst of 32x64 tiles (K=2 hi/lo rows per row-group).
                ps = psum_pool.tile([128, CHUNK // 2], f32)
                NB = CHUNK // (2 * MM_FREE)  # banks per group
                for w in range(2):  # stationary w0, then w1
                    for c in range(NB):
                        pa = ps[0:P, c * MM_FREE : (c + 1) * MM_FREE]
                        pb = ps[P:128, c * MM_FREE : (c + 1) * MM_FREE]
                        lo = slice(c * 2 * MM_FREE, c * 2 * MM_FREE + MM_FREE)
                        hi = slice(c * 2 * MM_FREE + MM_FREE, (c + 1) * 2 * MM_FREE)
                        nc.tensor.matmul(
                            pa, wt[:, w, :], bt[:, w, lo],
                            start=(w == 0), stop=False,
                        )
                        nc.tensor.matmul(
                            pb, wt[:, w, :], bt[:, w, hi],
                            start=(w == 0), stop=False, skip_group_check=True,
                        )
                for c in range(NB):
                    pa = ps[0:P, c * MM_FREE : (c + 1) * MM_FREE]
                    pb = ps[P:128, c * MM_FREE : (c + 1) * MM_FREE]
                    nlo = slice(g * NPG, g * NPG + MM_FREE)
                    nhi = slice(g * NPG + MM_FREE, g * NPG + 2 * MM_FREE)
                    nc.tensor.matmul(
                        pa, wneg[32 * c : 32 * c + 2, :],
                        nrm_sb[32 * c : 32 * c + 2, nlo],
                        start=False, stop=True, tile_position=(32 * c, 0),
                    )
                    nc.tensor.matmul(
                        pb, wneg[32 * c : 32 * c + 2, :],
                        nrm_sb[32 * c : 32 * c + 2, nhi],
                        start=False, stop=True, tile_position=(32 * c, 64),
                        skip_group_check=True,
                    )

                nc.vector.max(out=cand_sb[:, g * 8 : (g + 1) * 8], in_=ps[:])

            nc.gpsimd.dma_start(cand.ap(), cand_sb[:])

    if split:
        _split_waits(nc, mybir)
    return nc


def _split_waits(nc, mybir):
    """walrus (neuronxcc) accepts at most ONE sync wait per instruction;
    hoist extra waits onto same-engine NoOps inserted just before. HWDGE
    and SWDGE waits both execute at the issuing engine's sequencer, so
    this preserves ordering."""
    k = 0
    for f in nc.m.functions:
        for bb in f.blocks:
            insts = list(bb.instructions)
            new = []
            changed = False
            for ins in insts:
                si = getattr(ins, "sync_info", None)
                if si is not None and si.on_wait and len(si.on_wait) > 1:
                    changed = True
                    waits = list(si.on_wait)
                    for w in waits[:-1]:
                        k += 1
                        nop = mybir.InstNoOp(
                            name=f"WSPLIT-{k}", engine=ins.engine,
                            ins=[], outs=[], bass_nofuse=True,
                        )
                        nop.sync_info = mybir.SyncInfo(on_wait=[w], on_update=[])
                        new.append(nop)
                    ins.sync_info = mybir.SyncInfo(
                        on_wait=[waits[-1]], on_update=list(si.on_update)
                    )
                new.append(ins)
            if changed:
                try:
                    bb.instructions = new
                except Exception:
                    bb.instructions[:] = new


def get_nc():
    if "nc" not in _compiled:
        _compiled["nc"] = _build_bass()
    return _compiled["nc"]


def _prep_inputs(feats1, feats2, positive_pairs):
    f1 = np.asarray(feats1, dtype=np.float32).reshape(C, HW)
    f2 = np.asarray(feats2, dtype=np.float32).reshape(C, HW)
    idx = np.asarray(positive_pairs)[0, :, 0].astype(np.int64)
    sel = f1[:, idx]  # [C, P] fp32 anchors
    a_norms = np.sum(sel.astype(np.float64) ** 2, axis=0)  # [P]

    bdt_np = ml_dtypes.float8_e4m3 if FP8 else ml_dtypes.bfloat16
    w = np.empty((2, 128, P), dtype=np.float32)
    w[0] = 2.0 * sel[:128]
    w[1] = 2.0 * sel[128:]
    wmat = w.astype(bdt_np)

    b_norms = np.einsum("ij,ij->j", f2, f2).astype(np.float32)  # [HW]
    # hi/lo bf16 split so the K=2 norm pass carries ~fp32 precision
    n_hi = b_norms.astype(ml_dtypes.bfloat16)
    n_lo = (b_norms - n_hi.astype(np.float32)).astype(ml_dtypes.bfloat16)
    nrm2 = np.stack([n_hi, n_lo])  # [2, HW] bf16
    f2b = f2.astype(bdt_np)

    def nrm_rt(core):
        # row-tiled layout: partition 32r+k, pos g*1024 + cg*512 + t holds
        # nrm2[k, core_slice + g*4096 + (2r+cg)*512 + t]
        a = nrm2[:, core * NPC : (core + 1) * NPC]
        v = a.reshape(2, N_GROUPS, 4, 2, 512)  # k g r cg t
        out = np.zeros((8, NPC // 4), dtype=ml_dtypes.bfloat16)
        for r in range(4):
            for k in range(2):
                out[2 * r + k] = v[k, :, r, :, :].reshape(-1)
        return out

    in_maps = [
        {
            "f2s": np.ascontiguousarray(f2b[:, c * NPC : (c + 1) * NPC]),
            "wmat": wmat,
            "nrm": nrm_rt(c),
        }
        for c in range(N_CORES)
    ]
    return in_maps, a_norms


def _reduce_host(cands, a_norms):
    # cands: [128, N_CORES * NCAND] fp32 of neg_e = 2ab - ||b||^2 candidates;
    # anchor m's candidates live in PSUM rows m and m+64 (column-half tiling)
    merged = np.concatenate([cands[:P], cands[P:]], axis=1)  # [P, 2*...]
    d = a_norms[:, None] - merged.astype(np.float64)  # squared distances
    top5 = np.partition(d, 4, axis=1)[:, :5]
    top5 = np.maximum(top5, 0.0)
    return np.float32(np.mean(np.mean(top5, axis=1)))


def kernel(feats1, feats2, positive_pairs, **run_kwargs):
    from concourse.bass_utils import run_bass_kernel_spmd

    in_maps, a_norms = _prep_inputs(feats1, feats2, positive_pairs)
    nc = get_nc()
    res = run_bass_kernel_spmd(nc, in_maps, list(range(N_CORES)), **run_kwargs)
    cands = np.concatenate([r["cand"] for r in res.results], axis=1)
    out = _reduce_host(cands, a_norms)
    if run_kwargs:
        return out, res
    return out
